# revision 15
# baseline (speedup 1.0000x reference)
"""Trainium2 Bass kernel for nn_Attention_34351148434119 (8 NeuronCores).

Reference computation (faithful quirks included):
  q_proj = hid @ Wq; q, gate = split(q_proj)     # q is DEAD code downstream
  k = hid @ Wk; v = hid @ Wv                     # [B,KV,S,D]
  v = RoPE(v)  (k is NOT roped; q roped but unused)
  scores = (k @ v^T) * sqrt(D) + mask; attn = softmax_t(scores)   # per kv head
  out = (tile_G(attn @ v) * sigmoid(gate)) @ Wo

Sharding: core = b*4 + j  (b = batch, j = rank in 4-core batch group).
Per batch, S=2048 is split into 16 blocks of 128 rows; core j owns blocks
{j, 4+j, 8+j, 12+j} (slot k block = 4k+j) so every core has an identical
causal workload (uniform SPMD graph; per-core specialization only via
staged data).  v is shared within each batch group by two AllGathers
(d-major fp32 for scores, row-major bf16 for attn@v).

Precision: logits have sigma~105 (SCALING MULTIPLIES by sqrt(D)), so
softmax is near-argmax; the k/v-proj + scores chain runs in fp32 on PE.
gate / attn@v / out-proj run in bf16.
"""
import sys
import numpy as np

sys.path.insert(0, "/opt/trn_rl_repo")

B, S, HS = 2, 2048, 2048
H, KV, D = 16, 4, 128
G = H // KV
SCALING = float(D) ** 0.5
P = 128
NB = S // P            # 16 row blocks per batch
NCORES = 8
RANKS = 4              # cores per batch group
SLOTS = 4              # owned 128-row blocks per core
ROWS = SLOTS * P       # 512 rows per core
CHUNK = 512            # t-chunk = 4 t-tiles
NCHUNK = S // CHUNK    # 4
KT = HS // P           # 16 contraction tiles
NEG_THRESH = -1e8

_CACHE = {}


def _mask_classes(mask):
    """Classify each (s-slot k, t-chunk c) 512x512 region of the SxS mask.

    0 = skip (everything <= NEG_THRESH: contributes exact 0 after softmax)
    1 = plain (all zeros: no add needed)
    2 = add  (mixed: stage values and add on-chip)
    Slot k rows across all cores = blocks 4k..4k+3 = rows [512k, 512k+512).
    """
    cls = [[0] * NCHUNK for _ in range(SLOTS)]
    for k in range(SLOTS):
        for c in range(NCHUNK):
            reg = mask[512 * k:512 * (k + 1), 512 * c:512 * (c + 1)]
            if (reg <= NEG_THRESH).all():
                cls[k][c] = 0
            elif (reg == 0).all():
                cls[k][c] = 1
            else:
                cls[k][c] = 2
    ok = True
    for k in range(SLOTS):
        comp = [c for c in range(NCHUNK) if cls[k][c] != 0]
        # computed chunks must be a prefix starting at 0
        if comp != list(range(len(comp))) or 0 not in comp:
            ok = False
    if ok:
        # {k : chunk c computed} must be a suffix of slots for each c
        for c in range(NCHUNK):
            ks = [k for k in range(SLOTS) if cls[k][c] != 0]
            if ks != list(range(SLOTS - len(ks), SLOTS)):
                ok = False
    if not ok:
        # fully dense fallback: always correct for any mask
        cls = [[2] * NCHUNK for _ in range(SLOTS)]
    return cls


def _build(classes):
    from contextlib import ExitStack

    from concourse import bacc, mybir, tile
    from concourse.masks import make_identity

    f32 = mybir.dt.float32
    bf16 = mybir.dt.bfloat16
    Alu = mybir.AluOpType
    Act = mybir.ActivationFunctionType

    computed = [[c for c in range(NCHUNK) if classes[k][c] != 0] for k in range(SLOTS)]
    add_idx = {}
    for k in range(SLOTS):
        for c in range(NCHUNK):
            if classes[k][c] == 2:
                add_idx[(k, c)] = len(add_idx)
    n_add = max(len(add_idx), 1)

    nc = bacc.Bacc("TRN2", target_bir_lowering=False, debug=False,
                   num_devices=NCORES)

    hidT_d = nc.declare_dram_parameter("hidT", [HS, ROWS], f32, isOutput=False)
    wqg_d = nc.declare_dram_parameter("wqg", [HS, HS], f32, isOutput=False)
    wk_d = nc.declare_dram_parameter("wk", [HS, KV * D], f32, isOutput=False)
    wv_d = nc.declare_dram_parameter("wv", [HS, KV * D], f32, isOutput=False)
    wo_d = nc.declare_dram_parameter("wo", [HS, HS], f32, isOutput=False)
    cosT_d = nc.declare_dram_parameter("cosT", [D, ROWS], f32, isOutput=False)
    sinT_d = nc.declare_dram_parameter("sinT", [D, ROWS], f32, isOutput=False)
    mask_d = nc.declare_dram_parameter("maskst", [n_add, P, CHUNK], f32,
                                       isOutput=False)
    out_d = nc.declare_dram_parameter("out", [ROWS, HS], f32, isOutput=True)

    rg = [[0, 1, 2, 3], [4, 5, 6, 7]]

    with tile.TileContext(nc) as tc, ExitStack() as ctx:
        sb = ctx.enter_context(tc.tile_pool(name="sb", bufs=2))
        ps = ctx.enter_context(tc.tile_pool(name="ps", bufs=8, space="PSUM"))
        dram = ctx.enter_context(tc.tile_pool(name="dram", bufs=1, space="DRAM"))

        # ---- constants ----
        id_f32 = sb.tile([P, P], f32, tag="c_idf")
        id_bf = sb.tile([P, P], bf16, tag="c_idb")
        make_identity(nc, id_f32[:])
        make_identity(nc, id_bf[:])
        cosT = sb.tile([D, ROWS], f32, tag="c_cos")
        sinT = sb.tile([D, ROWS], f32, tag="c_sin")
        nc.sync.dma_start(cosT[:], cosT_d[:, :])
        nc.sync.dma_start(sinT[:], sinT_d[:, :])
        mstrip = []
        for i in range(len(add_idx)):
            t = sb.tile([P, CHUNK], f32, tag=f"c_msk{i}")
            nc.sync.dma_start(t[:], mask_d[i, :, :])
            mstrip.append(t)

        # ---- v projection FIRST (fp32) so the AllGathers trigger early ----
        pv = [ps.tile([P, ROWS], f32, tag="ps", name=f"pv{g}") for g in range(KV)]
        hidb = []
        for kk in range(KT):
            hf = sb.tile([P, ROWS], f32, tag="hidf", bufs=3)
            nc.sync.dma_start(hf[:], hidT_d[kk * P:(kk + 1) * P, :])
            wt = sb.tile([P, KV * D], f32, tag="wkv", bufs=2)
            nc.sync.dma_start(wt[:], wv_d[kk * P:(kk + 1) * P, :])
            for g in range(KV):
                nc.tensor.matmul(pv[g][:], wt[:, g * P:(g + 1) * P],
                                 hf[:], start=(kk == 0), stop=(kk == KT - 1))
            hb = sb.tile([P, ROWS], bf16, tag="bf16big", bufs=16)
            nc.scalar.copy(hb[:], hf[:])
            hidb.append(hb)

        vT = []   # per g: [128 d, 512 rows] f32, roped (in-place on vr)
        for g in range(KV):
            vr = sb.tile([P, ROWS], f32, tag="vraw", bufs=4)
            nc.scalar.copy(vr[:], pv[g][:])
            # RoPE: v' = v*cos + rot(v)*sin,  rot = [-v[64:], v[:64]]
            rot = sb.tile([P, ROWS], f32, tag="vrot", bufs=2)
            nc.vector.tensor_scalar_mul(rot[0:64, :], vr[64:128, :], -1.0)
            nc.vector.tensor_copy(rot[64:128, :], vr[0:64, :])
            nc.vector.tensor_mul(vr[:], vr[:], cosT[:])
            nc.vector.tensor_mul(rot[:], rot[:], sinT[:])
            nc.vector.tensor_add(vr[:], vr[:], rot[:])
            vT.append(vr)

        # ---- v row-major (bf16) via PE transpose ----
        vrow = []   # per rt: [128 rows, 512 d] bf16
        for rt in range(SLOTS):
            t = sb.tile([P, KV * D], bf16, tag="vrow", bufs=4)
            vrow.append(t)
        for g in range(KV):
            for rt in range(SLOTS):
                tp = ps.tile([P, P], f32, tag="ps")
                nc.tensor.transpose(tp[:], vT[g][:, rt * P:(rt + 1) * P], id_f32[:])
                nc.vector.tensor_copy(vrow[rt][:, g * P:(g + 1) * P], tp[:])

        # ---- AllGather v in both layouts (within 4-core batch group) ----
        vt_in = dram.tile([KV * D, ROWS], f32)
        vt_all_d = dram.tile([RANKS * KV * D, ROWS], f32)
        vr_in = dram.tile([ROWS, KV * D], bf16)
        vr_all_d = dram.tile([RANKS * ROWS, KV * D], bf16)
        for g in range(KV):
            nc.sync.dma_start(vt_in[g * P:(g + 1) * P, :], vT[g][:])
        for rt in range(SLOTS):
            nc.sync.dma_start(vr_in[rt * P:(rt + 1) * P, :], vrow[rt][:])
        nc.gpsimd.collective_compute(
            "AllGather", mybir.AluOpType.bypass, replica_groups=rg,
            ins=[vt_in.opt()], outs=[vt_all_d.opt()])
        nc.gpsimd.collective_compute(
            "AllGather", mybir.AluOpType.bypass, replica_groups=rg,
            ins=[vr_in.opt()], outs=[vr_all_d.opt()])

        # ---- k projection (fp32) — runs while the AllGathers are in flight ----
        pk = [ps.tile([P, ROWS], f32, tag="ps", name=f"pk{g}") for g in range(KV)]
        for kk in range(KT):
            hf = sb.tile([P, ROWS], f32, tag="hidf", bufs=3, name=f"hfk{kk}")
            nc.sync.dma_start(hf[:], hidT_d[kk * P:(kk + 1) * P, :])
            wt = sb.tile([P, KV * D], f32, tag="wkv", bufs=2, name=f"wtk{kk}")
            nc.sync.dma_start(wt[:], wk_d[kk * P:(kk + 1) * P, :])
            for g in range(KV):
                nc.tensor.matmul(pk[g][:], wt[:, g * P:(g + 1) * P],
                                 hf[:], start=(kk == 0), stop=(kk == KT - 1))
        kT = []   # per g: [128 d, 512 rows] f32, pre-scaled by sqrt(D)
        for g in range(KV):
            t = sb.tile([P, ROWS], f32, tag="kT", bufs=4)
            nc.scalar.mul(t[:], pk[g][:], SCALING)
            kT.append(t)

        # gathered tiles: index r*4+g -> [128 d(g), rank r's 512 rows] (fp32)
        #                 index r*4+x -> [128 rows(slot x of r), 512 d] (bf16)
        vtg = []
        for i in range(RANKS * KV):
            t = sb.tile([P, ROWS], f32, tag="f32big", bufs=16)
            nc.sync.dma_start(t[:], vt_all_d[i * P:(i + 1) * P, :])
            vtg.append(t)
        vrg = []
        for i in range(RANKS * SLOTS):
            t = sb.tile([P, KV * D], bf16, tag="bf16big", bufs=16)
            nc.sync.dma_start(t[:], vr_all_d[i * P:(i + 1) * P, :])
            vrg.append(t)

        # ---- gate matmul (bf16) + fused sigmoid ----
        sigT = []   # 16 tiles [128 gate-cols, 512 rows] bf16
        for nblk in range(4):
            wqb = []
            for kk in range(KT):
                fs = sb.tile([P, CHUNK], f32, tag="wslab", bufs=3)
                nc.sync.dma_start(
                    fs[:], wqg_d[kk * P:(kk + 1) * P, nblk * CHUNK:(nblk + 1) * CHUNK])
                bs = sb.tile([P, CHUNK], bf16, tag="wslabb", bufs=16)
                (nc.vector if kk % 2 else nc.scalar).tensor_copy(bs[:], fs[:]) if kk % 2 else nc.scalar.copy(bs[:], fs[:])
                wqb.append(bs)
            for m in range(4):
                pg = ps.tile([P, ROWS], f32, tag="ps")
                for kk in range(KT):
                    nc.tensor.matmul(pg[:], wqb[kk][:, m * P:(m + 1) * P],
                                     hidb[kk][:], start=(kk == 0), stop=(kk == KT - 1))
                t = sb.tile([P, ROWS], bf16, tag="sigT", bufs=16)
                nc.scalar.activation(t[:], pg[:], Act.Sigmoid)
                sigT.append(t)

        # ---- attention per kv head ----
        avT = []   # per g: [128 d, 512 rows] bf16
        for g in range(KV):
            # attnT tiles per t-block bi: [128 t, 512 s] bf16
            attnT = [sb.tile([P, ROWS], bf16, tag="attnT", bufs=16, name=f"attnT{g}_{bi}")
                     for bi in range(NB)]
            for k in range(SLOTS):
                comp = computed[k]
                nchk = len(comp)
                attn = sb.tile([P, CHUNK * nchk], bf16, tag=f"attn{k}", bufs=1)
                mtiles = []
                csums = []
                for ci, c in enumerate(comp):
                    psc = ps.tile([P, CHUNK], f32, tag="ps")
                    for i in range(4):
                        bi = 4 * c + i
                        rhs = vtg[(bi % RANKS) * KV + g][:, (bi // RANKS) * P:
                                                         (bi // RANKS) * P + P]
                        nc.tensor.matmul(psc[:, i * P:(i + 1) * P],
                                         kT[g][:, k * P:(k + 1) * P], rhs,
                                         start=True, stop=True)
                    if classes[k][c] == 2:
                        nc.vector.tensor_add(psc[:], psc[:],
                                             mstrip[add_idx[(k, c)]][:])
                    # negated running max chain (reduce negate=True -> -max)
                    cm = sb.tile([P, 1], f32, tag="stat", bufs=32)
                    nc.vector.tensor_reduce(cm[:], psc[:], mybir.AxisListType.X,
                                            Alu.max, negate=True)
                    if ci == 0:
                        mrun = cm
                    else:
                        mnew = sb.tile([P, 1], f32, tag="stat", bufs=32)
                        nc.vector.tensor_tensor(mnew[:], mrun[:], cm[:], Alu.min)
                        mrun = mnew
                    mtiles.append(mrun)   # holds -m_running
                    csum = sb.tile([P, 1], f32, tag="stat", bufs=32)
                    nc.scalar.activation(attn[:, ci * CHUNK:(ci + 1) * CHUNK],
                                         psc[:], Act.Exp, bias=mrun[:],
                                         accum_out=csum[:])
                    csums.append(csum)
                # finalize softmax: corrections + 1/sum
                negf = mtiles[-1]   # -m_final
                fcs = []
                tot = None
                for ci in range(nchk):
                    if ci == nchk - 1:
                        fcs.append(None)
                        if tot is None:
                            tot = csums[ci]
                        else:
                            t2 = sb.tile([P, 1], f32, tag="stat", bufs=32)
                            nc.vector.tensor_add(t2[:], tot[:], csums[ci][:])
                            tot = t2
                    else:
                        # f_c = exp(m_c - m_fin) = exp(-(-m_c) + (-m_fin))
                        fc = sb.tile([P, 1], f32, tag="stat", bufs=32)
                        nc.scalar.activation(fc[:], mtiles[ci][:], Act.Exp,
                                             bias=negf[:], scale=-1.0)
                        fcs.append(fc)
                        w = sb.tile([P, 1], f32, tag="stat", bufs=32)
                        nc.vector.tensor_mul(w[:], csums[ci][:], fc[:])
                        if tot is None:
                            tot = w
                        else:
                            t2 = sb.tile([P, 1], f32, tag="stat", bufs=32)
                            nc.vector.tensor_add(t2[:], tot[:], w[:])
                            tot = t2
                rinv = sb.tile([P, 1], f32, tag="stat", bufs=32)
                nc.vector.reciprocal(rinv[:], tot[:])
                for ci in range(nchk):
                    if fcs[ci] is None:
                        sc = rinv
                    else:
                        sc = sb.tile([P, 1], f32, tag="stat", bufs=32)
                        nc.vector.tensor_mul(sc[:], fcs[ci][:], rinv[:])
                    nc.vector.tensor_scalar_mul(
                        attn[:, ci * CHUNK:(ci + 1) * CHUNK],
                        attn[:, ci * CHUNK:(ci + 1) * CHUNK], sc[:])
                # transpose attn -> attnT column slot k
                for ci, c in enumerate(comp):
                    for i in range(4):
                        bi = 4 * c + i
                        tp = ps.tile([P, P], bf16, tag="ps")
                        nc.tensor.transpose(
                            tp[:], attn[:, ci * CHUNK + i * P:ci * CHUNK + (i + 1) * P],
                            id_bf[:])
                        nc.vector.tensor_copy(attnT[bi][:, k * P:(k + 1) * P], tp[:])
            # attn @ v  ->  avT[g] [128 d, 512 s]
            pav = ps.tile([P, ROWS], f32, tag="ps")
            first = True
            for bi in range(NB):
                ks = [k for k in range(SLOTS) if (bi // RANKS) in computed[k]]
                if not ks:
                    continue
                kmin = ks[0]
                lhs = vrg[(bi % RANKS) * SLOTS + (bi // RANKS)][:, g * P:(g + 1) * P]
                nc.tensor.matmul(pav[:, kmin * P:ROWS], lhs,
                                 attnT[bi][:, kmin * P:ROWS],
                                 start=first, stop=(bi == NB - 1))
                first = False
            t = sb.tile([P, ROWS], bf16, tag="avT", bufs=4)
            nc.vector.tensor_copy(t[:], pav[:])
            avT.append(t)

        # ---- gated = tile_G(avT) * sigT  (bf16) ----
        gat = []
        for g in range(KV):
            for i in range(G):
                t = sb.tile([P, ROWS], bf16, tag="gat", bufs=16)
                nc.vector.tensor_mul(t[:], avT[g][:], sigT[4 * g + i][:])
                gat.append(t)

        # ---- out projection (bf16) ----
        for nblk in range(4):
            wob = []
            for cc in range(KT):
                fs = sb.tile([P, CHUNK], f32, tag="wslab", bufs=3)
                nc.sync.dma_start(
                    fs[:], wo_d[cc * P:(cc + 1) * P, nblk * CHUNK:(nblk + 1) * CHUNK])
                bs = sb.tile([P, CHUNK], bf16, tag="wslabb", bufs=16)
                (nc.vector if cc % 2 else nc.scalar).tensor_copy(bs[:], fs[:]) if cc % 2 else nc.scalar.copy(bs[:], fs[:])
                wob.append(bs)
            for rt in range(SLOTS):
                po = ps.tile([P, CHUNK], f32, tag="ps")
                for cc in range(KT):
                    nc.tensor.matmul(po[:], gat[cc][:, rt * P:(rt + 1) * P],
                                     wob[cc][:], start=(cc == 0), stop=(cc == KT - 1))
                t = sb.tile([P, CHUNK], f32, tag="oev", bufs=2)
                nc.scalar.copy(t[:], po[:])
                nc.sync.dma_start(
                    out_d[rt * P:(rt + 1) * P, nblk * CHUNK:(nblk + 1) * CHUNK], t[:])

    nc.compile()
    return nc


def kernel(hidden_states, cos, sin, attention_mask, Wq, Wk, Wv, Wo):
    from concourse.bass_utils import run_bass_kernel_spmd

    hidden_states = np.asarray(hidden_states, dtype=np.float32)
    cos = np.asarray(cos, dtype=np.float32)
    sin = np.asarray(sin, dtype=np.float32)
    mask = np.asarray(attention_mask, dtype=np.float32)[0, 0]
    Wq = np.asarray(Wq, dtype=np.float32)
    Wk = np.asarray(Wk, dtype=np.float32)
    Wv = np.asarray(Wv, dtype=np.float32)
    Wo = np.asarray(Wo, dtype=np.float32)

    classes = _mask_classes(mask)
    key = tuple(tuple(r) for r in classes)
    if key not in _CACHE:
        _CACHE[key] = _build(classes)
    nc = _CACHE[key]

    add_strips = []   # staged per core below; order must match build
    wqg = np.ascontiguousarray(Wq[:, HS:])

    in_maps = []
    for core in range(NCORES):
        b, j = divmod(core, RANKS)
        blocks = [RANKS * k + j for k in range(SLOTS)]
        rows = np.concatenate([np.arange(bi * P, (bi + 1) * P) for bi in blocks])
        strips = []
        for k in range(SLOTS):
            for c in range(NCHUNK):
                if classes[k][c] == 2:
                    bi = RANKS * k + j
                    strips.append(mask[bi * P:(bi + 1) * P,
                                       c * CHUNK:(c + 1) * CHUNK])
        if not strips:
            strips.append(np.zeros((P, CHUNK), np.float32))
        in_maps.append({
            "hidT": np.ascontiguousarray(hidden_states[b][rows].T),
            "wqg": wqg,
            "wk": Wk,
            "wv": Wv,
            "wo": Wo,
            "cosT": np.ascontiguousarray(cos[b][rows].T),
            "sinT": np.ascontiguousarray(sin[b][rows].T),
            "maskst": np.ascontiguousarray(np.stack(strips)),
        })

    res = run_bass_kernel_spmd(nc, in_maps, core_ids=list(range(NCORES)))

    out = np.empty((B, S, HS), np.float32)
    for core in range(NCORES):
        b, j = divmod(core, RANKS)
        o = res.results[core]["out"]
        for k in range(SLOTS):
            bi = RANKS * k + j
            out[b, bi * P:(bi + 1) * P, :] = o[k * P:(k + 1) * P, :]
    return out


# revision 18
# speedup vs baseline: 1.0358x; 1.0358x over previous
"""Trainium2 Bass kernel for nn_Attention_34351148434119 (8 NeuronCores).

Reference computation (faithful quirks included):
  q_proj = hid @ Wq; q, gate = split(q_proj)     # q is DEAD code downstream
  k = hid @ Wk; v = hid @ Wv                     # [B,KV,S,D]
  v = RoPE(v)  (k is NOT roped; q roped but unused)
  scores = (k @ v^T) * sqrt(D) + mask; attn = softmax_t(scores)   # per kv head
  out = (tile_G(attn @ v) * sigmoid(gate)) @ Wo

Sharding: core = b*4 + j  (b = batch, j = rank in 4-core batch group).
Per batch, S=2048 is split into 16 blocks of 128 rows; core j owns blocks
{j, 4+j, 8+j, 12+j} (slot k block = 4k+j) so every core has an identical
causal workload (uniform SPMD graph; per-core specialization only via
staged data).  v is shared within each batch group by two AllGathers
(d-major fp32 for scores, row-major bf16 for attn@v).

Precision: logits have sigma~105 (SCALING MULTIPLIES by sqrt(D)), so
softmax is near-argmax; the k/v-proj + scores chain runs in fp32 on PE.
gate / attn@v / out-proj run in bf16.
"""
import sys
import numpy as np

sys.path.insert(0, "/opt/trn_rl_repo")

B, S, HS = 2, 2048, 2048
H, KV, D = 16, 4, 128
G = H // KV
SCALING = float(D) ** 0.5
P = 128
NB = S // P            # 16 row blocks per batch
NCORES = 8
RANKS = 4              # cores per batch group
SLOTS = 4              # owned 128-row blocks per core
ROWS = SLOTS * P       # 512 rows per core
CHUNK = 512            # t-chunk = 4 t-tiles
NCHUNK = S // CHUNK    # 4
KT = HS // P           # 16 contraction tiles
NEG_THRESH = -1e8

_CACHE = {}


def _mask_classes(mask):
    """Classify each (s-slot k, t-chunk c) 512x512 region of the SxS mask.

    0 = skip (everything <= NEG_THRESH: contributes exact 0 after softmax)
    1 = plain (all zeros: no add needed)
    2 = add  (mixed: stage values and add on-chip)
    Slot k rows across all cores = blocks 4k..4k+3 = rows [512k, 512k+512).
    """
    cls = [[0] * NCHUNK for _ in range(SLOTS)]
    for k in range(SLOTS):
        for c in range(NCHUNK):
            reg = mask[512 * k:512 * (k + 1), 512 * c:512 * (c + 1)]
            if (reg <= NEG_THRESH).all():
                cls[k][c] = 0
            elif (reg == 0).all():
                cls[k][c] = 1
            else:
                cls[k][c] = 2
    ok = True
    for k in range(SLOTS):
        comp = [c for c in range(NCHUNK) if cls[k][c] != 0]
        # computed chunks must be a prefix starting at 0
        if comp != list(range(len(comp))) or 0 not in comp:
            ok = False
    if ok:
        # {k : chunk c computed} must be a suffix of slots for each c
        for c in range(NCHUNK):
            ks = [k for k in range(SLOTS) if cls[k][c] != 0]
            if ks != list(range(SLOTS - len(ks), SLOTS)):
                ok = False
    if not ok:
        # fully dense fallback: always correct for any mask
        cls = [[2] * NCHUNK for _ in range(SLOTS)]
    return cls


def _build(classes):
    from contextlib import ExitStack

    from concourse import bacc, mybir, tile
    from concourse.masks import make_identity

    f32 = mybir.dt.float32
    bf16 = mybir.dt.bfloat16
    Alu = mybir.AluOpType
    Act = mybir.ActivationFunctionType

    computed = [[c for c in range(NCHUNK) if classes[k][c] != 0] for k in range(SLOTS)]
    add_idx = {}
    for k in range(SLOTS):
        for c in range(NCHUNK):
            if classes[k][c] == 2:
                add_idx[(k, c)] = len(add_idx)
    n_add = max(len(add_idx), 1)

    nc = bacc.Bacc("TRN2", target_bir_lowering=False, debug=False,
                   num_devices=NCORES)

    hidT_d = nc.declare_dram_parameter("hidT", [HS, ROWS], f32, isOutput=False)
    wqg_d = nc.declare_dram_parameter("wqg", [HS, HS], f32, isOutput=False)
    wk_d = nc.declare_dram_parameter("wk", [HS, KV * D], f32, isOutput=False)
    wv_d = nc.declare_dram_parameter("wv", [HS, KV * D], f32, isOutput=False)
    wo_d = nc.declare_dram_parameter("wo", [HS, HS], f32, isOutput=False)
    cosT_d = nc.declare_dram_parameter("cosT", [D, ROWS], f32, isOutput=False)
    sinT_d = nc.declare_dram_parameter("sinT", [D, ROWS], f32, isOutput=False)
    mask_d = nc.declare_dram_parameter("maskst", [n_add, P, CHUNK], f32,
                                       isOutput=False)
    out_d = nc.declare_dram_parameter("out", [ROWS, HS], f32, isOutput=True)

    rg = [[0, 1, 2, 3], [4, 5, 6, 7]]

    with tile.TileContext(nc) as tc, ExitStack() as ctx:
        sb = ctx.enter_context(tc.tile_pool(name="sb", bufs=2))
        ps = ctx.enter_context(tc.tile_pool(name="ps", bufs=8, space="PSUM"))
        dram = ctx.enter_context(tc.tile_pool(name="dram", bufs=1, space="DRAM"))

        # ---- constants ----
        id_f32 = sb.tile([P, P], f32, tag="c_idf")
        id_bf = sb.tile([P, P], bf16, tag="c_idb")
        make_identity(nc, id_f32[:])
        make_identity(nc, id_bf[:])
        cosT = sb.tile([D, ROWS], f32, tag="c_cos")
        sinT = sb.tile([D, ROWS], f32, tag="c_sin")
        nc.sync.dma_start(cosT[:], cosT_d[:, :])
        nc.sync.dma_start(sinT[:], sinT_d[:, :])
        mstrip = []
        for i in range(len(add_idx)):
            t = sb.tile([P, CHUNK], f32, tag=f"c_msk{i}")
            nc.sync.dma_start(t[:], mask_d[i, :, :])
            mstrip.append(t)

        # ---- v projection FIRST so the AllGathers trigger early ----
        # hi/lo bf16 split (3 passes: Whi*Hhi + Wlo*Hhi + Whi*Hlo) gives
        # ~16-bit effective precision at bf16 PE speed (fp32 is 4x slower).
        pv = [ps.tile([P, ROWS], f32, tag="ps", name=f"pv{g}") for g in range(KV)]
        hidb = []
        for kk in range(KT):
            hf = sb.tile([P, ROWS], f32, tag="hidf", bufs=3)
            nc.sync.dma_start(hf[:], hidT_d[kk * P:(kk + 1) * P, :])
            wt = sb.tile([P, KV * D], f32, tag="wkv", bufs=2)
            nc.sync.dma_start(wt[:], wv_d[kk * P:(kk + 1) * P, :])
            hb = sb.tile([P, ROWS], bf16, tag="bf16big", bufs=16)
            nc.scalar.copy(hb[:], hf[:])
            hlo = sb.tile([P, ROWS], bf16, tag="hlo", bufs=3)
            nc.vector.tensor_sub(hlo[:], hf[:], hb[:])
            whi = sb.tile([P, KV * D], bf16, tag="whi", bufs=2)
            nc.scalar.copy(whi[:], wt[:])
            wlo = sb.tile([P, KV * D], bf16, tag="wlo", bufs=2)
            nc.vector.tensor_sub(wlo[:], wt[:], whi[:])
            for g in range(KV):
                sl = slice(g * P, (g + 1) * P)
                nc.tensor.matmul(pv[g][:], whi[:, sl], hb[:],
                                 start=(kk == 0), stop=False)
                nc.tensor.matmul(pv[g][:], wlo[:, sl], hb[:],
                                 start=False, stop=False)
                nc.tensor.matmul(pv[g][:], whi[:, sl], hlo[:],
                                 start=False, stop=(kk == KT - 1))
            hidb.append(hb)

        vT = []   # per g: [128 d, 512 rows] f32, roped (in-place on vr)
        for g in range(KV):
            vr = sb.tile([P, ROWS], f32, tag="vraw", bufs=4)
            nc.scalar.copy(vr[:], pv[g][:])
            # RoPE: v' = v*cos + rot(v)*sin,  rot = [-v[64:], v[:64]]
            rot = sb.tile([P, ROWS], f32, tag="vrot", bufs=2)
            nc.vector.tensor_scalar_mul(rot[0:64, :], vr[64:128, :], -1.0)
            nc.vector.tensor_copy(rot[64:128, :], vr[0:64, :])
            nc.vector.tensor_mul(vr[:], vr[:], cosT[:])
            nc.vector.tensor_mul(rot[:], rot[:], sinT[:])
            nc.vector.tensor_add(vr[:], vr[:], rot[:])
            vT.append(vr)

        # ---- v row-major (bf16) via PE transpose ----
        vrow = []   # per rt: [128 rows, 512 d] bf16
        for rt in range(SLOTS):
            t = sb.tile([P, KV * D], bf16, tag="vrow", bufs=4)
            vrow.append(t)
        for g in range(KV):
            for rt in range(SLOTS):
                tp = ps.tile([P, P], f32, tag="ps")
                nc.tensor.transpose(tp[:], vT[g][:, rt * P:(rt + 1) * P], id_f32[:])
                nc.vector.tensor_copy(vrow[rt][:, g * P:(g + 1) * P], tp[:])

        # ---- AllGather v in both layouts (within 4-core batch group) ----
        vt_in = dram.tile([KV * D, ROWS], f32)
        vt_all_d = dram.tile([RANKS * KV * D, ROWS], f32)
        vr_in = dram.tile([ROWS, KV * D], bf16)
        vr_all_d = dram.tile([RANKS * ROWS, KV * D], bf16)
        for g in range(KV):
            nc.sync.dma_start(vt_in[g * P:(g + 1) * P, :], vT[g][:])
        for rt in range(SLOTS):
            nc.sync.dma_start(vr_in[rt * P:(rt + 1) * P, :], vrow[rt][:])
        nc.gpsimd.collective_compute(
            "AllGather", mybir.AluOpType.bypass, replica_groups=rg,
            ins=[vt_in.opt()], outs=[vt_all_d.opt()])
        nc.gpsimd.collective_compute(
            "AllGather", mybir.AluOpType.bypass, replica_groups=rg,
            ins=[vr_in.opt()], outs=[vr_all_d.opt()])

        # ---- k projection (hi/lo split) — runs while the AllGathers fly ----
        pk = [ps.tile([P, ROWS], f32, tag="ps", name=f"pk{g}") for g in range(KV)]
        for kk in range(KT):
            hf = sb.tile([P, ROWS], f32, tag="hidf", bufs=3, name=f"hfk{kk}")
            nc.sync.dma_start(hf[:], hidT_d[kk * P:(kk + 1) * P, :])
            wt = sb.tile([P, KV * D], f32, tag="wkv", bufs=2, name=f"wtk{kk}")
            nc.sync.dma_start(wt[:], wk_d[kk * P:(kk + 1) * P, :])
            hlo = sb.tile([P, ROWS], bf16, tag="hlo", bufs=3, name=f"hlok{kk}")
            nc.vector.tensor_sub(hlo[:], hf[:], hidb[kk][:])
            whi = sb.tile([P, KV * D], bf16, tag="whi", bufs=2, name=f"whik{kk}")
            nc.scalar.copy(whi[:], wt[:])
            wlo = sb.tile([P, KV * D], bf16, tag="wlo", bufs=2, name=f"wlok{kk}")
            nc.vector.tensor_sub(wlo[:], wt[:], whi[:])
            for g in range(KV):
                sl = slice(g * P, (g + 1) * P)
                nc.tensor.matmul(pk[g][:], whi[:, sl], hidb[kk][:],
                                 start=(kk == 0), stop=False)
                nc.tensor.matmul(pk[g][:], wlo[:, sl], hidb[kk][:],
                                 start=False, stop=False)
                nc.tensor.matmul(pk[g][:], whi[:, sl], hlo[:],
                                 start=False, stop=(kk == KT - 1))
        kT = []   # per g: [128 d, 512 rows] f32, pre-scaled by sqrt(D)
        for g in range(KV):
            t = sb.tile([P, ROWS], f32, tag="kT", bufs=4)
            nc.scalar.mul(t[:], pk[g][:], SCALING)
            kT.append(t)

        # ---- gate matmul (bf16) + fused sigmoid ----
        sigT = []   # 16 tiles [128 gate-cols, 512 rows] bf16
        for nblk in range(4):
            wqb = []
            for kk in range(KT):
                fs = sb.tile([P, CHUNK], f32, tag="wslab", bufs=2)
                nc.sync.dma_start(
                    fs[:], wqg_d[kk * P:(kk + 1) * P, nblk * CHUNK:(nblk + 1) * CHUNK])
                bs = sb.tile([P, CHUNK], bf16, tag="wslabb", bufs=16)
                (nc.vector if kk % 2 else nc.scalar).tensor_copy(bs[:], fs[:]) if kk % 2 else nc.scalar.copy(bs[:], fs[:])
                wqb.append(bs)
            for m in range(4):
                pg = ps.tile([P, ROWS], f32, tag="ps")
                for kk in range(KT):
                    nc.tensor.matmul(pg[:], wqb[kk][:, m * P:(m + 1) * P],
                                     hidb[kk][:], start=(kk == 0), stop=(kk == KT - 1))
                t = sb.tile([P, ROWS], bf16, tag="sigT", bufs=16)
                nc.scalar.activation(t[:], pg[:], Act.Sigmoid)
                sigT.append(t)

        # ---- load gathered v (gpsimd DMA queue, after gate weight DMAs) ----
        # vtc[g*NCHUNK+c]: [128 d(g), 512 t] f32, causal chunk c = blocks 4c..4c+3;
        # t-block 4c+r lives in rank r's AG chunk at column-slot c.
        vtc = []
        for g in range(KV):
            for c in range(NCHUNK):
                t = sb.tile([P, CHUNK], f32, tag="f32big", bufs=16,
                            name=f"vtc{g}_{c}")
                for r in range(RANKS):
                    nc.gpsimd.dma_start(
                        t[:, r * P:(r + 1) * P],
                        vt_all_d[r * KV * D + g * P:r * KV * D + (g + 1) * P,
                                 c * P:(c + 1) * P])
                vtc.append(t)
        vrg = []
        for i in range(RANKS * SLOTS):
            t = sb.tile([P, KV * D], bf16, tag="bf16big", bufs=16, name=f"vrg{i}")
            nc.gpsimd.dma_start(t[:], vr_all_d[i * P:(i + 1) * P, :])
            vrg.append(t)

        # ---- attention per kv head ----
        avT = []   # per g: [128 d, 512 rows] bf16
        for g in range(KV):
            # attnT tiles per t-block bi: [128 t, 512 s] bf16
            attnT = [sb.tile([P, ROWS], bf16, tag="attnT", bufs=16, name=f"attnT{g}_{bi}")
                     for bi in range(NB)]
            for k in range(SLOTS):
                comp = computed[k]
                nchk = len(comp)
                attn = sb.tile([P, CHUNK * nchk], bf16, tag=f"attn{k}", bufs=1)
                mtiles = []
                csums = []
                for ci, c in enumerate(comp):
                    psc = ps.tile([P, CHUNK], f32, tag="ps")
                    nc.tensor.matmul(psc[:], kT[g][:, k * P:(k + 1) * P],
                                     vtc[g * NCHUNK + c][:], start=True, stop=True)
                    if classes[k][c] == 2:
                        nc.vector.tensor_add(psc[:], psc[:],
                                             mstrip[add_idx[(k, c)]][:])
                    # negated running max chain (reduce negate=True -> -max)
                    cm = sb.tile([P, 1], f32, tag="stat", bufs=32)
                    nc.vector.tensor_reduce(cm[:], psc[:], mybir.AxisListType.X,
                                            Alu.max, negate=True)
                    if ci == 0:
                        mrun = cm
                    else:
                        mnew = sb.tile([P, 1], f32, tag="stat", bufs=32)
                        nc.vector.tensor_tensor(mnew[:], mrun[:], cm[:], Alu.min)
                        mrun = mnew
                    mtiles.append(mrun)   # holds -m_running
                    csum = sb.tile([P, 1], f32, tag="stat", bufs=32)
                    nc.scalar.activation(attn[:, ci * CHUNK:(ci + 1) * CHUNK],
                                         psc[:], Act.Exp, bias=mrun[:],
                                         accum_out=csum[:])
                    csums.append(csum)
                # finalize softmax: corrections + 1/sum
                negf = mtiles[-1]   # -m_final
                fcs = []
                tot = None
                for ci in range(nchk):
                    if ci == nchk - 1:
                        fcs.append(None)
                        if tot is None:
                            tot = csums[ci]
                        else:
                            t2 = sb.tile([P, 1], f32, tag="stat", bufs=32)
                            nc.vector.tensor_add(t2[:], tot[:], csums[ci][:])
                            tot = t2
                    else:
                        # f_c = exp(m_c - m_fin) = exp(-(-m_c) + (-m_fin))
                        fc = sb.tile([P, 1], f32, tag="stat", bufs=32)
                        nc.scalar.activation(fc[:], mtiles[ci][:], Act.Exp,
                                             bias=negf[:], scale=-1.0)
                        fcs.append(fc)
                        w = sb.tile([P, 1], f32, tag="stat", bufs=32)
                        nc.vector.tensor_mul(w[:], csums[ci][:], fc[:])
                        if tot is None:
                            tot = w
                        else:
                            t2 = sb.tile([P, 1], f32, tag="stat", bufs=32)
                            nc.vector.tensor_add(t2[:], tot[:], w[:])
                            tot = t2
                rinv = sb.tile([P, 1], f32, tag="stat", bufs=32)
                nc.vector.reciprocal(rinv[:], tot[:])
                for ci in range(nchk):
                    if fcs[ci] is None:
                        sc = rinv
                    else:
                        sc = sb.tile([P, 1], f32, tag="stat", bufs=32)
                        nc.vector.tensor_mul(sc[:], fcs[ci][:], rinv[:])
                    nc.vector.tensor_scalar_mul(
                        attn[:, ci * CHUNK:(ci + 1) * CHUNK],
                        attn[:, ci * CHUNK:(ci + 1) * CHUNK], sc[:])
                # transpose attn -> attnT column slot k
                for ci, c in enumerate(comp):
                    for i in range(4):
                        bi = 4 * c + i
                        tp = ps.tile([P, P], bf16, tag="ps")
                        nc.tensor.transpose(
                            tp[:], attn[:, ci * CHUNK + i * P:ci * CHUNK + (i + 1) * P],
                            id_bf[:])
                        nc.vector.tensor_copy(attnT[bi][:, k * P:(k + 1) * P], tp[:])
            # attn @ v  ->  avT[g] [128 d, 512 s]
            pav = ps.tile([P, ROWS], f32, tag="ps")
            first = True
            for bi in range(NB):
                ks = [k for k in range(SLOTS) if (bi // RANKS) in computed[k]]
                if not ks:
                    continue
                kmin = ks[0]
                lhs = vrg[(bi % RANKS) * SLOTS + (bi // RANKS)][:, g * P:(g + 1) * P]
                nc.tensor.matmul(pav[:, kmin * P:ROWS], lhs,
                                 attnT[bi][:, kmin * P:ROWS],
                                 start=first, stop=(bi == NB - 1))
                first = False
            t = sb.tile([P, ROWS], bf16, tag="avT", bufs=4)
            nc.vector.tensor_copy(t[:], pav[:])
            avT.append(t)

        # ---- gated = tile_G(avT) * sigT  (bf16) ----
        gat = []
        for g in range(KV):
            for i in range(G):
                t = sb.tile([P, ROWS], bf16, tag="gat", bufs=16)
                nc.vector.tensor_mul(t[:], avT[g][:], sigT[4 * g + i][:])
                gat.append(t)

        # ---- out projection (bf16) ----
        for nblk in range(4):
            wob = []
            for cc in range(KT):
                fs = sb.tile([P, CHUNK], f32, tag="wslab", bufs=2)
                nc.sync.dma_start(
                    fs[:], wo_d[cc * P:(cc + 1) * P, nblk * CHUNK:(nblk + 1) * CHUNK])
                bs = sb.tile([P, CHUNK], bf16, tag="wslabb", bufs=16)
                (nc.vector if cc % 2 else nc.scalar).tensor_copy(bs[:], fs[:]) if cc % 2 else nc.scalar.copy(bs[:], fs[:])
                wob.append(bs)
            for rt in range(SLOTS):
                po = ps.tile([P, CHUNK], f32, tag="ps")
                for cc in range(KT):
                    nc.tensor.matmul(po[:], gat[cc][:, rt * P:(rt + 1) * P],
                                     wob[cc][:], start=(cc == 0), stop=(cc == KT - 1))
                t = sb.tile([P, CHUNK], f32, tag="oev", bufs=2)
                nc.scalar.copy(t[:], po[:])
                nc.sync.dma_start(
                    out_d[rt * P:(rt + 1) * P, nblk * CHUNK:(nblk + 1) * CHUNK], t[:])

    nc.compile()
    return nc


def kernel(hidden_states, cos, sin, attention_mask, Wq, Wk, Wv, Wo):
    from concourse.bass_utils import run_bass_kernel_spmd

    hidden_states = np.asarray(hidden_states, dtype=np.float32)
    cos = np.asarray(cos, dtype=np.float32)
    sin = np.asarray(sin, dtype=np.float32)
    mask = np.asarray(attention_mask, dtype=np.float32)[0, 0]
    Wq = np.asarray(Wq, dtype=np.float32)
    Wk = np.asarray(Wk, dtype=np.float32)
    Wv = np.asarray(Wv, dtype=np.float32)
    Wo = np.asarray(Wo, dtype=np.float32)

    classes = _mask_classes(mask)
    key = tuple(tuple(r) for r in classes)
    if key not in _CACHE:
        _CACHE[key] = _build(classes)
    nc = _CACHE[key]

    add_strips = []   # staged per core below; order must match build
    wqg = np.ascontiguousarray(Wq[:, HS:])

    in_maps = []
    for core in range(NCORES):
        b, j = divmod(core, RANKS)
        blocks = [RANKS * k + j for k in range(SLOTS)]
        rows = np.concatenate([np.arange(bi * P, (bi + 1) * P) for bi in blocks])
        strips = []
        for k in range(SLOTS):
            for c in range(NCHUNK):
                if classes[k][c] == 2:
                    bi = RANKS * k + j
                    strips.append(mask[bi * P:(bi + 1) * P,
                                       c * CHUNK:(c + 1) * CHUNK])
        if not strips:
            strips.append(np.zeros((P, CHUNK), np.float32))
        in_maps.append({
            "hidT": np.ascontiguousarray(hidden_states[b][rows].T),
            "wqg": wqg,
            "wk": Wk,
            "wv": Wv,
            "wo": Wo,
            "cosT": np.ascontiguousarray(cos[b][rows].T),
            "sinT": np.ascontiguousarray(sin[b][rows].T),
            "maskst": np.ascontiguousarray(np.stack(strips)),
        })

    res = run_bass_kernel_spmd(nc, in_maps, core_ids=list(range(NCORES)))

    out = np.empty((B, S, HS), np.float32)
    for core in range(NCORES):
        b, j = divmod(core, RANKS)
        o = res.results[core]["out"]
        for k in range(SLOTS):
            bi = RANKS * k + j
            out[b, bi * P:(bi + 1) * P, :] = o[k * P:(k + 1) * P, :]
    return out
